# revision 7
# baseline (speedup 1.0000x reference)
"""CenterLoss kernel for Trainium2 (8 NeuronCores, SPMD data-parallel).

Reference computes
    distmat[b,c] = ||x_b||^2 + ||c_c||^2 - 2<x_b, c_c>          [B, C]
    loss = sum(clip(distmat * onehot(labels), 1e-12, 1e12)) / B

Only distmat[b, labels[b]] survives the mask; each of the B*(C-1) masked
zeros becomes exactly 1e-12 under the clip. So instead of the [8192, 10000]
distmat (42 GFLOP), each core gathers its rows' centers with indirect DMA
and computes per-row squared distances; the host adds the closed-form
constant B*(C-1)*1e-12 and divides by B.

Sharding: batch split 8 ways (1024 rows/core), centers replicated.

Per-core kernel (all stock ISA):
  - one [128, 8] int32 idx tile (labels, row p*8+g at [p, g])
  - one contiguous 1MB x load as [128, 8, 256] (row p*8+g at [p, g, :])
  - 8 indirect row-gathers (Q7 SWDGE, 128 rows each) whose offset APs are
    column slices of the idx tile; compute pipelined behind them:
    DVE subtract, ACT Square with accum_out giving the row reduction.
  - [128, 8] partial distances DMA'd out; host clamps at 1e-12 and sums.

Hard-won HW constraints baked in here (this runtime rejects/crashes
otherwise):
  - Use Bacc, and finalize() before run: TRN2 codegen allows ONE sync-wait
    per instruction; Bacc's generate_event_semaphores splits multi-waits,
    and the bass2jax path serializes the module without finalizing.
  - Stock instructions only: custom "Ant" ISA ops (tensor_tensor_reduce,
    dma_gather, ...) kill the exec unit (NRT_EXEC_UNIT_UNRECOVERABLE).
  - No in-place DVE ops (out aliasing an input) — same crash.
  - indirect_dma_start: offset AP may be a [128, 1] column slice, but the
    dest must be a whole [128, D] tile; multi-column offsets or strided
    dest slices gather garbage.
"""

import numpy as np

from concourse import bacc, bass, mybir
import concourse.tile as tile
from concourse.bass_utils import run_bass_kernel_spmd

B = 8192
C = 10000
D = 256
N_CORES = 8
BL = B // N_CORES  # rows per core
P = 128            # SBUF partitions
G = BL // P        # row groups per core

_CLIP_LO = 1e-12

_nc_cache = None


def _build():
    global _nc_cache
    if _nc_cache is not None:
        return _nc_cache

    nc = bacc.Bacc()
    x_l = nc.dram_tensor("x_local", [BL, D], mybir.dt.float32, kind="ExternalInput")
    lab_l = nc.dram_tensor("labels_local", [BL], mybir.dt.int32, kind="ExternalInput")
    cen = nc.dram_tensor("centers", [C, D], mybir.dt.float32, kind="ExternalInput")
    out = nc.dram_tensor("partials", [P, G], mybir.dt.float32, kind="ExternalOutput")

    with tile.TileContext(nc) as tc:
        with (
            tc.tile_pool(name="big", bufs=1) as big,
            tc.tile_pool(name="work", bufs=4) as work,
        ):
            lt = big.tile([P, G], mybir.dt.int32)
            xt = big.tile([P, G, D], mybir.dt.float32)
            acc = big.tile([P, G], mybir.dt.float32)

            # idx tile first: the whole gather chain hangs off it
            nc.sync.dma_start(out=lt[:], in_=lab_l[:].rearrange("(p g) -> p g", g=G))
            # x in halves so early groups aren't gated on the full 1MB
            x_ap = x_l[:].rearrange("(p g) d -> p g d", g=G)
            nc.sync.dma_start(out=xt[:, 0:G // 2, :], in_=x_ap[:, 0:G // 2, :])
            nc.sync.dma_start(out=xt[:, G // 2:, :], in_=x_ap[:, G // 2:, :])

            for g in range(G):
                ct = work.tile([P, D], mybir.dt.float32, tag="ct")
                nc.gpsimd.indirect_dma_start(
                    out=ct[:],
                    out_offset=None,
                    in_=cen[:],
                    in_offset=bass.IndirectOffsetOnAxis(ap=lt[:, g:g + 1], axis=0),
                )
                dt = work.tile([P, D], mybir.dt.float32, tag="dt")
                nc.vector.tensor_sub(out=dt[:], in0=xt[:, g, :], in1=ct[:])
                sq = work.tile([P, D], mybir.dt.float32, tag="sq")
                nc.scalar.activation(
                    out=sq[:],
                    in_=dt[:],
                    func=mybir.ActivationFunctionType.Square,
                    accum_out=acc[:, g:g + 1],
                )
            nc.sync.dma_start(out=out[:], in_=acc[:])

    nc.finalize()
    _nc_cache = nc
    return nc


def _run(x, labels, centers, **spmd_kwargs):
    nc = _build()
    x = np.ascontiguousarray(np.asarray(x), dtype=np.float32)
    labels = np.ascontiguousarray(np.asarray(labels)).astype(np.int32)
    centers = np.ascontiguousarray(np.asarray(centers), dtype=np.float32)

    in_maps = []
    for c in range(N_CORES):
        sl = slice(c * BL, (c + 1) * BL)
        in_maps.append(
            {
                "x_local": x[sl],
                "labels_local": labels[sl],
                "centers": centers,
            }
        )
    res = run_bass_kernel_spmd(nc, in_maps, list(range(N_CORES)), **spmd_kwargs)
    partials = np.stack([r["partials"] for r in res.results])  # [8, P, G]
    clamped = np.maximum(partials.astype(np.float64), _CLIP_LO)
    loss = (clamped.sum() + B * (C - 1) * _CLIP_LO) / B
    return np.asarray(loss, dtype=np.float32), res


def kernel(x, labels, centers):
    loss, _ = _run(x, labels, centers)
    return loss


# revision 9
# speedup vs baseline: 1.1308x; 1.1308x over previous
"""CenterLoss kernel for Trainium2 (8 NeuronCores, SPMD data-parallel).

Reference computes
    distmat[b,c] = ||x_b||^2 + ||c_c||^2 - 2<x_b, c_c>          [B, C]
    loss = sum(clip(distmat * onehot(labels), 1e-12, 1e12)) / B

Only distmat[b, labels[b]] survives the mask; each of the B*(C-1) masked
zeros becomes exactly 1e-12 under the clip. So instead of the [8192, 10000]
distmat (42 GFLOP), each core gathers its rows' centers with indirect DMA
and computes per-row squared distances; the host adds the closed-form
constant B*(C-1)*1e-12 and divides by B.

Sharding: batch split 8 ways (1024 rows/core), centers replicated.

Per-core kernel (all stock ISA):
  - one [128, 8] int32 idx tile (labels, row p*8+g at [p, g])
  - one contiguous 1MB x load as [128, 8, 256] (row p*8+g at [p, g, :])
  - 8 indirect row-gathers (Q7 SWDGE, 128 rows each) whose offset APs are
    column slices of the idx tile; compute pipelined behind them:
    DVE subtract, ACT Square with accum_out giving the row reduction.
  - [128, 8] partial distances DMA'd out; host clamps at 1e-12 and sums.

Hard-won HW constraints baked in here (this runtime rejects/crashes
otherwise):
  - Use Bacc, and finalize() before run: TRN2 codegen allows ONE sync-wait
    per instruction; Bacc's generate_event_semaphores splits multi-waits,
    and the bass2jax path serializes the module without finalizing.
  - Stock instructions only: custom "Ant" ISA ops (tensor_tensor_reduce,
    dma_gather, ...) kill the exec unit (NRT_EXEC_UNIT_UNRECOVERABLE).
  - No in-place DVE ops (out aliasing an input) — same crash.
  - indirect_dma_start: offset AP may be a [128, 1] column slice, but the
    dest must be a whole [128, D] tile; multi-column offsets or strided
    dest slices gather garbage.
"""

import numpy as np

from concourse import bacc, bass, mybir
import concourse.tile as tile
from concourse.bass_utils import run_bass_kernel_spmd

B = 8192
C = 10000
D = 256
N_CORES = 8
BL = B // N_CORES  # rows per core
P = 128            # SBUF partitions
G = BL // P        # row groups per core

_CLIP_LO = 1e-12

_nc_cache = None


def _build():
    global _nc_cache
    if _nc_cache is not None:
        return _nc_cache

    nc = bacc.Bacc()
    x_l = nc.dram_tensor("x_local", [BL, D], mybir.dt.float32, kind="ExternalInput")
    lab_l = nc.dram_tensor("labels_local", [BL], mybir.dt.int32, kind="ExternalInput")
    cen = nc.dram_tensor("centers", [C, D], mybir.dt.float32, kind="ExternalInput")
    out = nc.dram_tensor("partials", [P, G], mybir.dt.float32, kind="ExternalOutput")

    with tile.TileContext(nc) as tc:
        with (
            tc.tile_pool(name="big", bufs=1) as big,
            tc.tile_pool(name="work", bufs=4) as work,
            # gather dests get all 8 slots: late gathers then never carry a
            # slot-release wait, keeping the Q7 chain free of EVSEM stalls
            tc.tile_pool(name="ctp", bufs=G) as ctp,
        ):
            lt = big.tile([P, G], mybir.dt.int32)
            xt = big.tile([P, G, D], mybir.dt.float32)
            acc = big.tile([P, G], mybir.dt.float32)

            # idx tile first: the whole gather chain hangs off it
            nc.sync.dma_start(out=lt[:], in_=lab_l[:].rearrange("(p g) -> p g", g=G))
            # x in halves so early groups aren't gated on the full 1MB
            x_ap = x_l[:].rearrange("(p g) d -> p g d", g=G)
            nc.sync.dma_start(out=xt[:, 0:G // 2, :], in_=x_ap[:, 0:G // 2, :])
            nc.sync.dma_start(out=xt[:, G // 2:, :], in_=x_ap[:, G // 2:, :])

            for g in range(G):
                ct = ctp.tile([P, D], mybir.dt.float32, tag="ct")
                nc.gpsimd.indirect_dma_start(
                    out=ct[:],
                    out_offset=None,
                    in_=cen[:],
                    in_offset=bass.IndirectOffsetOnAxis(ap=lt[:, g:g + 1], axis=0),
                )
                dt = work.tile([P, D], mybir.dt.float32, tag="dt")
                nc.vector.tensor_sub(out=dt[:], in0=xt[:, g, :], in1=ct[:])
                sq = work.tile([P, D], mybir.dt.float32, tag="sq")
                nc.scalar.activation(
                    out=sq[:],
                    in_=dt[:],
                    func=mybir.ActivationFunctionType.Square,
                    accum_out=acc[:, g:g + 1],
                )
            nc.sync.dma_start(out=out[:], in_=acc[:])

    nc.finalize()
    _nc_cache = nc
    return nc


def _run(x, labels, centers, **spmd_kwargs):
    nc = _build()
    x = np.ascontiguousarray(np.asarray(x), dtype=np.float32)
    labels = np.ascontiguousarray(np.asarray(labels)).astype(np.int32)
    centers = np.ascontiguousarray(np.asarray(centers), dtype=np.float32)

    in_maps = []
    for c in range(N_CORES):
        sl = slice(c * BL, (c + 1) * BL)
        in_maps.append(
            {
                "x_local": x[sl],
                "labels_local": labels[sl],
                "centers": centers,
            }
        )
    res = run_bass_kernel_spmd(nc, in_maps, list(range(N_CORES)), **spmd_kwargs)
    partials = np.stack([r["partials"] for r in res.results])  # [8, P, G]
    clamped = np.maximum(partials.astype(np.float64), _CLIP_LO)
    loss = (clamped.sum() + B * (C - 1) * _CLIP_LO) / B
    return np.asarray(loss, dtype=np.float32), res


def kernel(x, labels, centers):
    loss, _ = _run(x, labels, centers)
    return loss
